# revision 14
# baseline (speedup 1.0000x reference)
"""Causal self-attention (S=8192, D=2048, DKQ=DV=128, fp32) on 8 Trainium2 cores.

Strategy (sequence-parallel, causal-balanced):
- 64 query tiles of 128 rows. Core c owns 8 tiles: for pair p in 0..3 it gets
  global tiles gA = 8p + c (needs few key columns) and gB = 63 - 8p - c (needs
  many), so every core does identical work (one compiled program, SPMD).
- Q/K projections and the score matmuls run in exact fp32 (4 cyc/row): the
  softmax here is near-one-hot (scores ~1e5, temperature ~1/11), so fp32r's
  TF32-like rounding (~1e-4 rel) flips argmax rows. V stays fp32r; the exp'd
  attention weights, transposes and PV matmuls run in fp16.
- Collectives overlap compute: K is projected first and AllGathered (fp32)
  behind the V/Q projections; V is gathered in fp16 (half the bytes) behind
  the Q projection and the first score chunks.
- Phase C groups slots of equal extent class ((1,3),(5,7),(4,6),(0,2)) so the
  per-pair zero padding is at most 2 chunks. Per chunk: PE matmul -> ACT
  copies PSUM->SBUF stage while DVE computes the chunk max from PSUM; exp on
  ACT (per-row bias, fused row-sum) writes the fp16 exp buffer. PV: fp16 PE
  transposes pack two slots into [128,256] moving operands at full rate.
"""

import os
import sys

for _p in ("/opt/trn_rl_repo", "/root/.axon_site/_ro/trn_rl_repo"):
    if os.path.isdir(_p) and _p not in sys.path:
        sys.path.append(_p)

import numpy as np

import concourse.bass as bass
import concourse.mybir as mybir
import concourse.tile as tile
from concourse import bacc
from concourse.bass_utils import run_bass_kernel_spmd
from concourse.masks import make_identity

P = 128
S = 8192
D = 2048
DK = 128
DV = 128
NCORES = 8
NSLOT = 8
MB = NSLOT * P  # rows per core
SCALE = 1.0 / float(np.sqrt(128.0))
NEG = -1.0e30
# slot s belongs to pair p = s//2; even (A) slots compute 2p+2 score chunks of
# 512 columns, odd (B) slots 16-2p.
C_SLOT = [2, 16, 4, 14, 6, 12, 8, 10]
# phase-C groups of two same-class slots (slotA, slotB, padded extent E),
# processed big-first so the V AllGather hides behind the first group's scores.
GROUPS = [(1, 3, 16), (5, 7, 12), (6, 4, 8), (2, 0, 4)]

f32 = mybir.dt.float32
f32r = mybir.dt.float32r
f16 = mybir.dt.float16


def _slot_to_g(c, s):
    p = s // 2
    return 8 * p + c if s % 2 == 0 else 63 - 8 * p - c


def _g_to_rank_slot(g):
    if g < 32:
        return g % 8, 2 * (g // 8)
    return (63 - g) % 8, 2 * ((63 - g) // 8) + 1


def _build_nc():
    nc = bacc.Bacc(
        "TRN2", target_bir_lowering=False, debug=False, num_devices=NCORES
    )
    xt = nc.dram_tensor("xt", [D, MB], f32, kind="ExternalInput").ap()
    msk = nc.dram_tensor("mask", [NSLOT, 2, P, 512], f32, kind="ExternalInput").ap()
    wq = nc.dram_tensor("wq", [D, DK], f32, kind="ExternalInput").ap()
    wk = nc.dram_tensor("wk", [D, DK], f32, kind="ExternalInput").ap()
    wv = nc.dram_tensor("wv", [D, DV], f32, kind="ExternalInput").ap()
    out = nc.dram_tensor("out", [MB, DV], f32, kind="ExternalOutput").ap()

    cc_k_in = nc.dram_tensor("cc_k_in", [P, MB], f32)
    cc_k_out = nc.dram_tensor("cc_k_out", [NCORES * P, MB], f32, addr_space="Shared")
    cc_v_in = nc.dram_tensor("cc_v_in", [MB, DV], f16)
    cc_v_out = nc.dram_tensor("cc_v_out", [S, DV], f16, addr_space="Shared")

    AX = mybir.AxisListType
    OP = mybir.AluOpType
    ACT = mybir.ActivationFunctionType

    with tile.TileContext(nc) as tc:
        with (
            tc.tile_pool(name="const", bufs=1) as const_pool,
            tc.tile_pool(name="resident", bufs=1) as res_pool,
        ):
            id32 = const_pool.tile([P, P], f32)
            make_identity(nc, id32[:])
            id16 = const_pool.tile([P, P], f16)
            make_identity(nc, id16[:])

            qt = res_pool.tile([P, MB], f32)  # Q^T, my rows, slot-ordered
            kt_sb = res_pool.tile([P, S], f32)  # full K^T
            vnat = res_pool.tile([P, 64, DV], f16)  # full V as 64 [s,dv] tiles
            rinv = res_pool.tile([P, NSLOT], f32)  # per-slot 1/rowsum

            # ---------- phase A: projections; K gathered first, V in fp16 ----
            with (
                tc.tile_pool(name="pa_sb", bufs=2) as pa_sb,
                tc.tile_pool(name="pa_w", bufs=1) as pa_w,
                tc.tile_pool(name="pa_keep", bufs=1) as pa_keep,
                tc.tile_pool(name="pa_ps", bufs=2, space="PSUM") as pa_ps,
            ):
                wkt = pa_w.tile([P, 16, DK], f32, tag="wk")
                nc.sync.dma_start(out=wkt[:], in_=wk.rearrange("(t p) d -> p t d", p=P))
                wvt = pa_w.tile([P, 16, DV], f32, tag="wv")
                nc.sync.dma_start(out=wvt[:], in_=wv.rearrange("(t p) d -> p t d", p=P))
                wqt = pa_w.tile([P, 16, DK], f32, tag="wq")
                nc.sync.dma_start(out=wqt[:], in_=wq.rearrange("(t p) d -> p t d", p=P))

                xt_r = xt.rearrange("(t p) m -> p t m", p=P)
                xc0 = pa_sb.tile([P, 16, 512], f32, tag="xc")
                nc.sync.dma_start(out=xc0[:], in_=xt_r[:, :, 0:512])
                xc1 = pa_sb.tile([P, 16, 512], f32, tag="xc")
                nc.sync.dma_start(out=xc1[:], in_=xt_r[:, :, 512:1024])
                xcs = (xc0, xc1)

                rg = [list(range(NCORES))]

                # K first: its (bigger, fp32) gather hides behind V+Q work.
                kt_mine = pa_keep.tile([P, MB], f32, tag="ktm")
                for m2 in range(2):
                    kp = pa_ps.tile([P, 512], f32, tag="kp")
                    for kk in range(16):
                        nc.tensor.matmul(
                            kp[:], wkt[:, kk, :], xcs[m2][:, kk, :],
                            start=(kk == 0), stop=(kk == 15),
                        )
                    nc.vector.tensor_copy(kt_mine[:, bass.ts(m2, 512)], kp[:])
                nc.sync.dma_start(out=cc_k_in[:], in_=kt_mine[:])
                nc.gpsimd.collective_compute(
                    "AllGather", OP.bypass, replica_groups=rg,
                    ins=[cc_k_in[:]], outs=[cc_k_out[:]],
                )

                # V next (fp32r matmuls), gathered as fp16.
                vt_mine = pa_keep.tile([P, MB], f32, tag="vtm")
                for m2 in range(2):
                    vp = pa_ps.tile([P, 512], f32, tag="vp")
                    for kk in range(16):
                        nc.tensor.matmul(
                            vp[:], wvt[:, kk, :], xcs[m2][:, kk, :],
                            start=(kk == 0), stop=(kk == 15),
                        )
                    nc.vector.tensor_copy(vt_mine[:, bass.ts(m2, 512)], vp[:])
                vnat_mine = pa_keep.tile([P, NSLOT, DV], f16, tag="vnm")
                for t in range(NSLOT):
                    vtp = pa_ps.tile([P, P], f32, tag="vtp")
                    nc.tensor.transpose(vtp[:], vt_mine[:, bass.ts(t, P)], id32[:])
                    nc.vector.tensor_copy(vnat_mine[:, t, :], vtp[:])
                nc.sync.dma_start(
                    out=cc_v_in.rearrange("(t p) d -> p t d", p=P),
                    in_=vnat_mine[:],
                )
                nc.gpsimd.collective_compute(
                    "AllGather", OP.bypass, replica_groups=rg,
                    ins=[cc_v_in[:]], outs=[cc_v_out[:]],
                )

                # Q last: purely local, overlaps both gathers.
                for m2 in range(2):
                    qp = pa_ps.tile([P, 512], f32, tag="qp")
                    for kk in range(16):
                        nc.tensor.matmul(
                            qp[:], wqt[:, kk, :], xcs[m2][:, kk, :],
                            start=(kk == 0), stop=(kk == 15),
                        )
                    nc.vector.tensor_copy(qt[:, bass.ts(m2, 512)], qp[:])

            # ---------- phase B: load gathered K^T and V ----------
            for g in range(64):
                r, s = _g_to_rank_slot(g)
                nc.sync.dma_start(
                    out=kt_sb[:, bass.ts(g, P)],
                    in_=cc_k_out[r * P : (r + 1) * P, s * P : (s + 1) * P],
                )
                nc.sync.dma_start(
                    out=vnat[:, g, :],
                    in_=cc_v_out[r * MB + s * P : r * MB + (s + 1) * P, :],
                )

            # ---------- phase C: attention ----------
            with (
                tc.tile_pool(name="stg", bufs=2) as stage_pool,
                tc.tile_pool(name="eb", bufs=3) as expb_pool,
                tc.tile_pool(name="mpool", bufs=4) as mpool,
                tc.tile_pool(name="stats", bufs=8) as stats,
                tc.tile_pool(name="atp", bufs=3) as atpool,
                tc.tile_pool(name="otp", bufs=2) as otpool,
                tc.tile_pool(name="osb", bufs=2) as osb_pool,
                tc.tile_pool(name="sps", bufs=3, space="PSUM") as spsum,
                tc.tile_pool(name="tps", bufs=2, space="PSUM") as tpsum,
                tc.tile_pool(name="ops", bufs=2, space="PSUM") as opsum,
            ):
                for sA, sB, E in GROUPS:
                    expb = {}
                    for slot in (sA, sB):
                        Cs = C_SLOT[slot]
                        stg = stage_pool.tile([P, Cs * 512], f32, tag="stg")
                        eb = expb_pool.tile([P, E * 512], f16, tag="eb")
                        expb[slot] = eb
                        cmax = stats.tile([P, 16], f32, tag="cmax")
                        for n in range(Cs):
                            sps = spsum.tile([P, 512], f32, tag="sps")
                            nc.tensor.matmul(
                                sps[:],
                                qt[:, bass.ts(slot, P)],
                                kt_sb[:, bass.ts(n, 512)],
                                start=True,
                                stop=True,
                            )
                            if n >= Cs - 2:
                                msb = mpool.tile([P, 512], f32, tag="msb")
                                nc.sync.dma_start(
                                    out=msb[:], in_=msk[slot, n - (Cs - 2)]
                                )
                                nc.vector.tensor_add(
                                    stg[:, bass.ts(n, 512)], sps[:], msb[:]
                                )
                                nc.vector.tensor_reduce(
                                    cmax[:, n : n + 1],
                                    stg[:, bass.ts(n, 512)],
                                    axis=AX.X,
                                    op=OP.max,
                                )
                            else:
                                nc.scalar.copy(stg[:, bass.ts(n, 512)], sps[:])
                                nc.vector.tensor_reduce(
                                    cmax[:, n : n + 1],
                                    sps[:],
                                    axis=AX.X,
                                    op=OP.max,
                                )
                        rmax = stats.tile([P, 1], f32, tag="rmax")
                        nc.vector.tensor_reduce(
                            rmax[:], cmax[:, :Cs], axis=AX.X, op=OP.max
                        )
                        negb = stats.tile([P, 1], f32, tag="negb")
                        nc.vector.tensor_scalar_mul(negb[:], rmax[:], -SCALE)
                        lsum = stats.tile([P, 1], f32, tag="lsum")
                        nc.scalar.activation(
                            out=eb[:, : Cs * 512],
                            in_=stg[:, : Cs * 512],
                            func=ACT.Exp,
                            bias=negb[:],
                            scale=SCALE,
                            accum_out=lsum[:],
                        )
                        nc.vector.reciprocal(rinv[:, slot : slot + 1], lsum[:])
                        if Cs < E:
                            nc.gpsimd.memset(eb[:, Cs * 512 :], 0.0)

                    ebA, ebB = expb[sA], expb[sB]
                    ops = opsum.tile([P, 256], f32, tag="ops")
                    for j in range(4 * E):
                        tps = tpsum.tile([P, 256], f16, tag="tps")
                        nc.tensor.transpose(
                            tps[:, :P], ebA[:, bass.ts(j, P)], id16[:]
                        )
                        nc.tensor.transpose(
                            tps[:, P:], ebB[:, bass.ts(j, P)], id16[:]
                        )
                        at = atpool.tile([P, 256], f16, tag="at")
                        if j % 2 == 0:
                            nc.vector.tensor_copy(at[:], tps[:])
                        else:
                            nc.scalar.copy(at[:], tps[:])
                        nc.tensor.matmul(
                            ops[:],
                            vnat[:, j, :],
                            at[:],
                            start=(j == 0),
                            stop=(j == 4 * E - 1),
                        )

                    ot = otpool.tile([P, 256], f32, tag="ot")
                    nc.vector.tensor_copy(ot[:], ops[:])
                    # ep tiles share the ops tag: the accumulator has been
                    # drained into ot by now, so the 2-buf rotation is safe.
                    for qq, slot in enumerate((sA, sB)):
                        ep = opsum.tile([P, 256], f32, tag="ops")
                        nc.tensor.transpose(
                            ep[:, :P], ot[:, bass.ts(qq, P)], id32[:]
                        )
                        o_sb = osb_pool.tile([P, DV], f32, tag="osb")
                        nc.vector.tensor_scalar_mul(
                            o_sb[:], ep[:, :P], rinv[:, slot : slot + 1]
                        )
                        nc.sync.dma_start(
                            out=out[bass.ts(slot, P), :], in_=o_sb[:]
                        )

    nc.compile()
    return nc


_NC_CACHE = None


def _get_nc():
    global _NC_CACHE
    if _NC_CACHE is None:
        _NC_CACHE = _build_nc()
    return _NC_CACHE


def _make_masks(c):
    m = np.zeros((NSLOT, 2, P, 512), dtype=np.float32)
    cols512 = np.arange(512)[None, :]
    rows128 = np.arange(P)[:, None]
    for s in range(NSLOT):
        g = _slot_to_g(c, s)
        C = C_SLOT[s]
        rows = g * P + rows128
        for jj in range(2):
            cols = (C - 2 + jj) * 512 + cols512
            m[s, jj] = np.where(cols <= rows, 0.0, NEG)
    return m


def _prep_in_maps(x, w_q, w_k, w_v):
    x = np.ascontiguousarray(x, dtype=np.float32)
    w_q = np.ascontiguousarray(w_q, dtype=np.float32)
    w_k = np.ascontiguousarray(w_k, dtype=np.float32)
    w_v = np.ascontiguousarray(w_v, dtype=np.float32)
    in_maps = []
    for c in range(NCORES):
        rows = np.concatenate(
            [
                np.arange(_slot_to_g(c, s) * P, (_slot_to_g(c, s) + 1) * P)
                for s in range(NSLOT)
            ]
        )
        in_maps.append(
            {
                "xt": np.ascontiguousarray(x[rows].T),
                "mask": _make_masks(c),
                "wq": w_q,
                "wk": w_k,
                "wv": w_v,
            }
        )
    return in_maps


def _run(x, w_q, w_k, w_v, trace=False, trace_cores=None):
    nc = _get_nc()
    in_maps = _prep_in_maps(x, w_q, w_k, w_v)
    res = run_bass_kernel_spmd(
        nc,
        in_maps,
        list(range(NCORES)),
        trace=trace,
        trace_cores=trace_cores,
    )
    out = np.zeros((S, DV), dtype=np.float32)
    for c in range(NCORES):
        oc = res.results[c]["out"]
        for s in range(NSLOT):
            g = _slot_to_g(c, s)
            out[g * P : (g + 1) * P] = oc[s * P : (s + 1) * P]
    return out, res


def kernel(**inputs):
    out, _ = _run(inputs["x"], inputs["w_q"], inputs["w_k"], inputs["w_v"])
    return out


# revision 23
# speedup vs baseline: 1.1149x; 1.1149x over previous
"""Causal self-attention (S=8192, D=2048, DKQ=DV=128, fp32) on 8 Trainium2 cores.

Strategy (sequence-parallel, causal-balanced):
- 64 query tiles of 128 rows. Core c owns 8 tiles: for pair p in 0..3 it gets
  global tiles gA = 8p + c (needs few key columns) and gB = 63 - 8p - c (needs
  many), so every core does identical work (one compiled program, SPMD).
- Q/K projections and the score matmuls run in exact fp32 (4 cyc/row): the
  softmax here is near-one-hot (scores ~1e5, temperature ~1/11), so fp32r's
  TF32-like rounding (~1e-4 rel) flips argmax rows. V stays fp32r; the exp'd
  attention weights, transposes and PV matmuls run in fp16.
- Collectives overlap compute: K is projected first and AllGathered (fp32)
  behind the V/Q projections; V is gathered in fp16 (half the bytes) behind
  the Q projection and the first score chunks.
- Phase C groups slots of equal extent class ((1,3),(5,7),(4,6),(0,2)) so the
  per-pair zero padding is at most 2 chunks. Per chunk: PE matmul -> ACT
  copies PSUM->SBUF stage while DVE computes the chunk max from PSUM; exp on
  ACT (per-row bias, fused row-sum) writes the fp16 exp buffer. PV: fp16 PE
  transposes pack two slots into [128,256] moving operands at full rate.
"""

import os
import sys

for _p in ("/opt/trn_rl_repo", "/root/.axon_site/_ro/trn_rl_repo"):
    if os.path.isdir(_p) and _p not in sys.path:
        sys.path.append(_p)

import numpy as np

import concourse.bass as bass
import concourse.mybir as mybir
import concourse.tile as tile
from concourse import bacc
from concourse.bass_utils import run_bass_kernel_spmd
from concourse.masks import make_identity

P = 128
S = 8192
D = 2048
DK = 128
DV = 128
NCORES = 8
NSLOT = 8
MB = NSLOT * P  # rows per core
SCALE = 1.0 / float(np.sqrt(128.0))
NEG = -1.0e30
# slot s belongs to pair p = s//2; even (A) slots compute 2p+2 score chunks of
# 512 columns, odd (B) slots 16-2p.
C_SLOT = [2, 16, 4, 14, 6, 12, 8, 10]
# phase-C groups of two same-class slots (slotA, slotB, padded extent E),
# processed big-first so the V AllGather hides behind the first group's scores.
GROUPS = [(1, 3, 16), (5, 7, 12), (6, 4, 8), (2, 0, 4)]

f32 = mybir.dt.float32
f32r = mybir.dt.float32r
f16 = mybir.dt.float16


def _slot_to_g(c, s):
    p = s // 2
    return 8 * p + c if s % 2 == 0 else 63 - 8 * p - c


def _g_to_rank_slot(g):
    if g < 32:
        return g % 8, 2 * (g // 8)
    return (63 - g) % 8, 2 * ((63 - g) // 8) + 1


def _build_nc():
    nc = bacc.Bacc(
        "TRN2", target_bir_lowering=False, debug=False, num_devices=NCORES
    )
    xt = nc.dram_tensor("xt", [D, MB], f32, kind="ExternalInput").ap()
    msk = nc.dram_tensor("mask", [NSLOT, P, 2, 512], f32, kind="ExternalInput").ap()
    wq = nc.dram_tensor("wq", [D, DK], f32, kind="ExternalInput").ap()
    wk = nc.dram_tensor("wk", [D, DK], f32, kind="ExternalInput").ap()
    wv = nc.dram_tensor("wv", [D, DV], f32, kind="ExternalInput").ap()
    out = nc.dram_tensor("out", [MB, DV], f32, kind="ExternalOutput").ap()

    cc_k_in = nc.dram_tensor("cc_k_in", [P, MB], f32)
    cc_k_out = nc.dram_tensor("cc_k_out", [NCORES * P, MB], f32, addr_space="Shared")
    cc_v_in = nc.dram_tensor("cc_v_in", [MB, DV], f16)
    cc_v_out = nc.dram_tensor("cc_v_out", [S, DV], f16, addr_space="Shared")

    AX = mybir.AxisListType
    OP = mybir.AluOpType
    ACT = mybir.ActivationFunctionType

    with tile.TileContext(nc) as tc:
        with (
            tc.tile_pool(name="const", bufs=1) as const_pool,
            tc.tile_pool(name="resident", bufs=1) as res_pool,
        ):
            id32 = const_pool.tile([P, P], f32)
            make_identity(nc, id32[:])
            id16 = const_pool.tile([P, P], f16)
            make_identity(nc, id16[:])

            qt = res_pool.tile([P, MB], f32)  # Q^T, my rows, slot-ordered
            kt_sb = res_pool.tile([P, S], f32)  # full K^T
            vnat = res_pool.tile([P, 64, DV], f16)  # full V as 64 [s,dv] tiles
            rinv = res_pool.tile([P, NSLOT], f32)  # per-slot 1/rowsum

            # ---------- phase A: projections; K gathered first, V in fp16 ----
            with (
                tc.tile_pool(name="pa_sb", bufs=2) as pa_sb,
                tc.tile_pool(name="pa_w", bufs=1) as pa_w,
                tc.tile_pool(name="pa_keep", bufs=1) as pa_keep,
                tc.tile_pool(name="pa_ps", bufs=2, space="PSUM") as pa_ps,
            ):
                wkt = pa_w.tile([P, 16, DK], f32, tag="wk")
                nc.sync.dma_start(out=wkt[:], in_=wk.rearrange("(t p) d -> p t d", p=P))
                wvt = pa_w.tile([P, 16, DV], f32, tag="wv")
                nc.sync.dma_start(out=wvt[:], in_=wv.rearrange("(t p) d -> p t d", p=P))
                wqt = pa_w.tile([P, 16, DK], f32, tag="wq")
                nc.sync.dma_start(out=wqt[:], in_=wq.rearrange("(t p) d -> p t d", p=P))

                # split the big loads into parallel pieces: a single
                # dma_start runs ~110 GB/s on one queue.
                xt_r = xt.rearrange("(t p) m -> p t m", p=P)
                xc0 = pa_sb.tile([P, 16, 512], f32, tag="xc")
                for t4 in range(4):
                    nc.sync.dma_start(
                        out=xc0[:, 4 * t4 : 4 * t4 + 4, :],
                        in_=xt_r[:, 4 * t4 : 4 * t4 + 4, 0:512],
                    )
                xc1 = pa_sb.tile([P, 16, 512], f32, tag="xc")
                for t4 in range(4):
                    nc.sync.dma_start(
                        out=xc1[:, 4 * t4 : 4 * t4 + 4, :],
                        in_=xt_r[:, 4 * t4 : 4 * t4 + 4, 512:1024],
                    )
                xcs = (xc0, xc1)

                rg = [list(range(NCORES))]

                # K first: its (bigger, fp32) gather hides behind V+Q work.
                kt_mine = pa_keep.tile([P, MB], f32, tag="ktm")
                for m2 in range(2):
                    kp = pa_ps.tile([P, 512], f32, tag="kp")
                    for kk in range(16):
                        nc.tensor.matmul(
                            kp[:], wkt[:, kk, :], xcs[m2][:, kk, :],
                            start=(kk == 0), stop=(kk == 15),
                        )
                    nc.vector.tensor_copy(kt_mine[:, bass.ts(m2, 512)], kp[:])
                for t8 in range(8):
                    nc.sync.dma_start(
                        out=cc_k_in[:, bass.ts(t8, P)],
                        in_=kt_mine[:, bass.ts(t8, P)],
                    )
                nc.gpsimd.collective_compute(
                    "AllGather", OP.bypass, replica_groups=rg,
                    ins=[cc_k_in[:]], outs=[cc_k_out[:]],
                )

                # V next (fp32r matmuls), gathered as fp16.
                vt_mine = pa_keep.tile([P, MB], f32, tag="vtm")
                for m2 in range(2):
                    vp = pa_ps.tile([P, 512], f32, tag="vp")
                    for kk in range(16):
                        nc.tensor.matmul(
                            vp[:], wvt[:, kk, :], xcs[m2][:, kk, :],
                            start=(kk == 0), stop=(kk == 15),
                        )
                    nc.vector.tensor_copy(vt_mine[:, bass.ts(m2, 512)], vp[:])
                vnat_mine = pa_keep.tile([P, NSLOT, DV], f16, tag="vnm")
                for t in range(NSLOT):
                    vtp = pa_ps.tile([P, P], f32, tag="vtp")
                    nc.tensor.transpose(vtp[:], vt_mine[:, bass.ts(t, P)], id32[:])
                    nc.vector.tensor_copy(vnat_mine[:, t, :], vtp[:])
                cc_v_r = cc_v_in.rearrange("(t p) d -> p t d", p=P)
                for t4 in range(4):
                    nc.sync.dma_start(
                        out=cc_v_r[:, 2 * t4 : 2 * t4 + 2, :],
                        in_=vnat_mine[:, 2 * t4 : 2 * t4 + 2, :],
                    )
                nc.gpsimd.collective_compute(
                    "AllGather", OP.bypass, replica_groups=rg,
                    ins=[cc_v_in[:]], outs=[cc_v_out[:]],
                )

                # Q last: purely local, overlaps both gathers.
                for m2 in range(2):
                    qp = pa_ps.tile([P, 512], f32, tag="qp")
                    for kk in range(16):
                        nc.tensor.matmul(
                            qp[:], wqt[:, kk, :], xcs[m2][:, kk, :],
                            start=(kk == 0), stop=(kk == 15),
                        )
                    nc.vector.tensor_copy(qt[:, bass.ts(m2, 512)], qp[:])

            # ---------- phase B: load gathered K^T and V ----------
            for g in range(64):
                r, s = _g_to_rank_slot(g)
                nc.sync.dma_start(
                    out=kt_sb[:, bass.ts(g, P)],
                    in_=cc_k_out[r * P : (r + 1) * P, s * P : (s + 1) * P],
                )
                nc.sync.dma_start(
                    out=vnat[:, g, :],
                    in_=cc_v_out[r * MB + s * P : r * MB + (s + 1) * P, :],
                )

            # ---------- phase C: attention ----------
            with (
                tc.tile_pool(name="stg", bufs=2) as stage_pool,
                tc.tile_pool(name="eb", bufs=3) as expb_pool,
                tc.tile_pool(name="mpool", bufs=3) as mpool,
                tc.tile_pool(name="stats", bufs=8) as stats,
                tc.tile_pool(name="atp", bufs=3) as atpool,
                tc.tile_pool(name="otp", bufs=2) as otpool,
                tc.tile_pool(name="osb", bufs=2) as osb_pool,
                tc.tile_pool(name="sps", bufs=2, space="PSUM") as spsum,
                tc.tile_pool(name="tps", bufs=2, space="PSUM") as tpsum,
                tc.tile_pool(name="ops", bufs=2, space="PSUM") as opsum,
            ):
                for sA, sB, E in GROUPS:
                    expb = {}
                    for slot in (sA, sB):
                        # score chunks in pairs: one 2-bank PSUM tile, one
                        # wide ACT copy (or DVE masked add), one dual-max.
                        Cs = C_SLOT[slot]
                        stg = stage_pool.tile([P, 16, 512], f32, tag="stg")
                        eb = expb_pool.tile([P, 16, 512], f16, tag="eb")
                        expb[slot] = eb
                        cmax = stats.tile([P, 16], f32, tag="cmax")
                        for h in range(Cs // 2):
                            n = 2 * h
                            sps = spsum.tile([P, 2, 512], f32, tag="sps")
                            for dn in range(2):
                                nc.tensor.matmul(
                                    sps[:, dn, :],
                                    qt[:, bass.ts(slot, P)],
                                    kt_sb[:, bass.ts(n + dn, 512)],
                                    start=True,
                                    stop=True,
                                )
                            if n == Cs - 2:
                                msb = mpool.tile([P, 2, 512], f32, tag="msb")
                                nc.sync.dma_start(out=msb[:], in_=msk[slot])
                                nc.vector.tensor_add(
                                    stg[:, n : n + 2, :], sps[:], msb[:]
                                )
                                nc.vector.tensor_reduce(
                                    cmax[:, n : n + 2],
                                    stg[:, n : n + 2, :],
                                    axis=AX.X,
                                    op=OP.max,
                                )
                            else:
                                nc.scalar.copy(stg[:, n : n + 2, :], sps[:])
                                nc.vector.tensor_reduce(
                                    cmax[:, n : n + 2],
                                    sps[:],
                                    axis=AX.X,
                                    op=OP.max,
                                )
                        rmax = stats.tile([P, 1], f32, tag="rmax")
                        nc.vector.tensor_reduce(
                            rmax[:], cmax[:, :Cs], axis=AX.X, op=OP.max
                        )
                        negb = stats.tile([P, 1], f32, tag="negb")
                        nc.vector.tensor_scalar_mul(negb[:], rmax[:], -SCALE)
                        lsum = stats.tile([P, 1], f32, tag="lsum")
                        nc.scalar.activation(
                            out=eb[:, :Cs, :],
                            in_=stg[:, :Cs, :],
                            func=ACT.Exp,
                            bias=negb[:],
                            scale=SCALE,
                            accum_out=lsum[:],
                        )
                        nc.vector.reciprocal(rinv[:, slot : slot + 1], lsum[:])
                        if Cs < E:
                            nc.gpsimd.memset(eb[:, Cs:E, :], 0.0)

                    ebA, ebB = expb[sA], expb[sB]

                    def ebt(eb, j):
                        return eb[:, j // 4, (j % 4) * P : (j % 4 + 1) * P]

                    ops = opsum.tile([P, 256], f32, tag="ops")
                    for j2 in range(2 * E):
                        j = 2 * j2
                        tps = tpsum.tile([P, 4, P], f16, tag="tps")
                        nc.tensor.transpose(tps[:, 0, :], ebt(ebA, j), id16[:])
                        nc.tensor.transpose(tps[:, 1, :], ebt(ebB, j), id16[:])
                        nc.tensor.transpose(
                            tps[:, 2, :], ebt(ebA, j + 1), id16[:]
                        )
                        nc.tensor.transpose(
                            tps[:, 3, :], ebt(ebB, j + 1), id16[:]
                        )
                        at = atpool.tile([P, 4, P], f16, tag="at")
                        if j2 % 2 == 0:
                            nc.vector.tensor_copy(at[:], tps[:])
                        else:
                            nc.scalar.copy(at[:], tps[:])
                        nc.tensor.matmul(
                            ops[:],
                            vnat[:, j, :],
                            at[:, 0:2, :],
                            start=(j == 0),
                            stop=False,
                        )
                        nc.tensor.matmul(
                            ops[:],
                            vnat[:, j + 1, :],
                            at[:, 2:4, :],
                            start=False,
                            stop=(j + 2 == 4 * E),
                        )

                    ot = otpool.tile([P, 256], f32, tag="ot")
                    nc.vector.tensor_copy(ot[:], ops[:])
                    # ep tiles share the ops tag: the accumulator has been
                    # drained into ot by now, so the 2-buf rotation is safe.
                    for qq, slot in enumerate((sA, sB)):
                        ep = opsum.tile([P, 256], f32, tag="ops")
                        nc.tensor.transpose(
                            ep[:, :P], ot[:, bass.ts(qq, P)], id32[:]
                        )
                        o_sb = osb_pool.tile([P, DV], f32, tag="osb")
                        nc.vector.tensor_scalar_mul(
                            o_sb[:], ep[:, :P], rinv[:, slot : slot + 1]
                        )
                        nc.sync.dma_start(
                            out=out[bass.ts(slot, P), :], in_=o_sb[:]
                        )

    nc.compile()
    return nc


_NC_CACHE = None


def _get_nc():
    global _NC_CACHE
    if _NC_CACHE is None:
        _NC_CACHE = _build_nc()
    return _NC_CACHE


def _make_masks(c):
    m = np.zeros((NSLOT, P, 2, 512), dtype=np.float32)
    cols512 = np.arange(512)[None, :]
    rows128 = np.arange(P)[:, None]
    for s in range(NSLOT):
        g = _slot_to_g(c, s)
        C = C_SLOT[s]
        rows = g * P + rows128
        for jj in range(2):
            cols = (C - 2 + jj) * 512 + cols512
            m[s, :, jj, :] = np.where(cols <= rows, 0.0, NEG)
    return m


def _prep_in_maps(x, w_q, w_k, w_v):
    x = np.ascontiguousarray(x, dtype=np.float32)
    w_q = np.ascontiguousarray(w_q, dtype=np.float32)
    w_k = np.ascontiguousarray(w_k, dtype=np.float32)
    w_v = np.ascontiguousarray(w_v, dtype=np.float32)
    in_maps = []
    for c in range(NCORES):
        rows = np.concatenate(
            [
                np.arange(_slot_to_g(c, s) * P, (_slot_to_g(c, s) + 1) * P)
                for s in range(NSLOT)
            ]
        )
        in_maps.append(
            {
                "xt": np.ascontiguousarray(x[rows].T),
                "mask": _make_masks(c),
                "wq": w_q,
                "wk": w_k,
                "wv": w_v,
            }
        )
    return in_maps


def _run(x, w_q, w_k, w_v, trace=False, trace_cores=None):
    nc = _get_nc()
    in_maps = _prep_in_maps(x, w_q, w_k, w_v)
    res = run_bass_kernel_spmd(
        nc,
        in_maps,
        list(range(NCORES)),
        trace=trace,
        trace_cores=trace_cores,
    )
    out = np.zeros((S, DV), dtype=np.float32)
    for c in range(NCORES):
        oc = res.results[c]["out"]
        for s in range(NSLOT):
            g = _slot_to_g(c, s)
            out[g * P : (g + 1) * P] = oc[s * P : (s + 1) * P]
    return out, res


def kernel(**inputs):
    out, _ = _run(inputs["x"], inputs["w_q"], inputs["w_k"], inputs["w_v"])
    return out


# revision 31
# speedup vs baseline: 1.4136x; 1.2680x over previous
"""Causal self-attention (S=8192, D=2048, DKQ=DV=128, fp32) on 8 Trainium2 cores.

Strategy (sequence-parallel, causal-balanced):
- 64 query tiles of 128 rows. Core c owns 8 tiles: for pair p in 0..3 it gets
  global tiles gA = 8p + c (needs few key columns) and gB = 63 - 8p - c (needs
  many), so every core does identical work (one compiled program, SPMD).
- Q/K projections and the score matmuls run in exact fp32 (4 cyc/row): the
  softmax here is near-one-hot (scores ~1e5, temperature ~1/11), so fp32r's
  TF32-like rounding (~1e-4 rel) flips argmax rows. V stays fp32r; the exp'd
  attention weights, transposes and PV matmuls run in fp16.
- Collectives overlap compute: K is projected first and AllGathered (fp32)
  behind the V/Q projections; V is gathered in fp16 (half the bytes) behind
  the Q projection and the first score chunks.
- Phase C groups slots of equal extent class ((1,3),(5,7),(4,6),(0,2)) so the
  per-pair zero padding is at most 2 chunks. Per chunk: PE matmul -> ACT
  copies PSUM->SBUF stage while DVE computes the chunk max from PSUM; exp on
  ACT (per-row bias, fused row-sum) writes the fp16 exp buffer. PV: fp16 PE
  transposes pack two slots into [128,256] moving operands at full rate.
"""

import os
import sys

for _p in ("/opt/trn_rl_repo", "/root/.axon_site/_ro/trn_rl_repo"):
    if os.path.isdir(_p) and _p not in sys.path:
        sys.path.append(_p)

import numpy as np

import concourse.bass as bass
import concourse.mybir as mybir
import concourse.tile as tile
from concourse import bacc
from concourse.bass_utils import run_bass_kernel_spmd
from concourse.masks import make_identity

P = 128
S = 8192
D = 2048
DK = 128
DV = 128
NCORES = 8
NSLOT = 8
MB = NSLOT * P  # rows per core
SCALE = 1.0 / float(np.sqrt(128.0))
NEG = -1.0e30
# slot s belongs to pair p = s//2; even (A) slots compute 2p+2 score chunks of
# 512 columns, odd (B) slots 16-2p.
C_SLOT = [2, 16, 4, 14, 6, 12, 8, 10]
# phase-C groups of two same-class slots (slotA, slotB, padded extent E),
# processed big-first so the V AllGather hides behind the first group's scores.
GROUPS = [(1, 3, 16), (5, 7, 12), (6, 4, 8), (2, 0, 4)]

f32 = mybir.dt.float32
f32r = mybir.dt.float32r
f16 = mybir.dt.float16


def _slot_to_g(c, s):
    # even slots: g = 8p + c; odd slots: g = 56 - 8p + c. With this map every
    # 512-col key chunk covers 4 consecutive ascending ranks of one slot, and
    # the causal mask on the last two chunks of ANY slot reduces to
    # "col_in_chunk <= p + 128*c" — one mask per core for all slots.
    p = s // 2
    return 8 * p + c if s % 2 == 0 else 56 - 8 * p + c


def _g_to_rank_slot(g):
    if g < 32:
        return g % 8, 2 * (g // 8)
    p = (63 - g) // 8
    return g - (56 - 8 * p), 2 * p + 1


def _build_nc():
    nc = bacc.Bacc(
        "TRN2", target_bir_lowering=False, debug=False, num_devices=NCORES
    )
    xt = nc.dram_tensor("xt", [D, MB], f32, kind="ExternalInput").ap()
    msk = nc.dram_tensor("mask", [P, 2, 512], f32, kind="ExternalInput").ap()
    wq = nc.dram_tensor("wq", [D, DK], f32, kind="ExternalInput").ap()
    wk = nc.dram_tensor("wk", [D, DK], f32, kind="ExternalInput").ap()
    wv = nc.dram_tensor("wv", [D, DV], f32, kind="ExternalInput").ap()
    out = nc.dram_tensor("out", [MB, DV], f32, kind="ExternalOutput").ap()

    cc_k_in = nc.dram_tensor("cc_k_in", [P, MB], f32)
    cc_k_out = nc.dram_tensor("cc_k_out", [NCORES * P, MB], f32, addr_space="Shared")
    cc_v_in = nc.dram_tensor("cc_v_in", [MB, DV], f16)
    cc_v_out = nc.dram_tensor("cc_v_out", [S, DV], f16, addr_space="Shared")

    AX = mybir.AxisListType
    OP = mybir.AluOpType
    ACT = mybir.ActivationFunctionType

    with tile.TileContext(nc) as tc:
        with (
            tc.tile_pool(name="const", bufs=1) as const_pool,
            tc.tile_pool(name="resident", bufs=1) as res_pool,
        ):
            id32 = const_pool.tile([P, P], f32)
            make_identity(nc, id32[:])
            id16 = const_pool.tile([P, P], f16)
            make_identity(nc, id16[:])

            qt = res_pool.tile([P, MB], f32)  # Q^T, my rows, slot-ordered
            kt_sb = res_pool.tile([P, S], f32)  # full K^T
            vnat = res_pool.tile([P, 64, DV], f16)  # full V as 64 [s,dv] tiles
            rinv = res_pool.tile([P, NSLOT], f32)  # per-slot 1/rowsum
            mask_sb = res_pool.tile([P, 2, 512], f32)  # shared causal mask
            nc.sync.dma_start(out=mask_sb[:], in_=msk)

            # ---------- phase A: projections; K gathered first, V in fp16 ----
            with (
                tc.tile_pool(name="pa_sb", bufs=2) as pa_sb,
                tc.tile_pool(name="pa_w", bufs=1) as pa_w,
                tc.tile_pool(name="pa_keep", bufs=1) as pa_keep,
                tc.tile_pool(name="pa_ps", bufs=2, space="PSUM") as pa_ps,
            ):
                # split the big loads into parallel pieces: a single
                # dma_start runs ~110 GB/s on one queue.
                wkt = pa_w.tile([P, 16, DK], f32, tag="wk")
                wk_r = wk.rearrange("(t p) d -> p t d", p=P)
                nc.sync.dma_start(out=wkt[:, :8, :], in_=wk_r[:, :8, :])
                nc.sync.dma_start(out=wkt[:, 8:, :], in_=wk_r[:, 8:, :])

                xt_r = xt.rearrange("(t p) m -> p t m", p=P)
                xc0 = pa_sb.tile([P, 16, 512], f32, tag="xc")
                for t2 in range(8):
                    nc.sync.dma_start(
                        out=xc0[:, 2 * t2 : 2 * t2 + 2, :],
                        in_=xt_r[:, 2 * t2 : 2 * t2 + 2, 0:512],
                    )
                xc1 = pa_sb.tile([P, 16, 512], f32, tag="xc")
                for t2 in range(8):
                    nc.sync.dma_start(
                        out=xc1[:, 2 * t2 : 2 * t2 + 2, :],
                        in_=xt_r[:, 2 * t2 : 2 * t2 + 2, 512:1024],
                    )
                xcs = (xc0, xc1)

                wvt = pa_w.tile([P, 16, DV], f32, tag="wv")
                wv_r = wv.rearrange("(t p) d -> p t d", p=P)
                nc.sync.dma_start(out=wvt[:, :8, :], in_=wv_r[:, :8, :])
                nc.sync.dma_start(out=wvt[:, 8:, :], in_=wv_r[:, 8:, :])
                wqt = pa_w.tile([P, 16, DK], f32, tag="wq")
                wq_r = wq.rearrange("(t p) d -> p t d", p=P)
                nc.sync.dma_start(out=wqt[:, :8, :], in_=wq_r[:, :8, :])
                nc.sync.dma_start(out=wqt[:, 8:, :], in_=wq_r[:, 8:, :])

                rg = [list(range(NCORES))]

                # K first: its (bigger, fp32) gather hides behind V+Q work.
                kt_mine = pa_keep.tile([P, MB], f32, tag="ktm")
                for m2 in range(2):
                    kp = pa_ps.tile([P, 512], f32, tag="kp")
                    for kk in range(16):
                        nc.tensor.matmul(
                            kp[:], wkt[:, kk, :], xcs[m2][:, kk, :],
                            start=(kk == 0), stop=(kk == 15),
                        )
                    nc.vector.tensor_copy(kt_mine[:, bass.ts(m2, 512)], kp[:])
                for t8 in range(8):
                    nc.sync.dma_start(
                        out=cc_k_in[:, bass.ts(t8, P)],
                        in_=kt_mine[:, bass.ts(t8, P)],
                    )
                nc.gpsimd.collective_compute(
                    "AllGather", OP.bypass, replica_groups=rg,
                    ins=[cc_k_in[:]], outs=[cc_k_out[:]],
                )

                # V next (fp32r matmuls), gathered as fp16.
                vt_mine = pa_keep.tile([P, MB], f32, tag="vtm")
                for m2 in range(2):
                    vp = pa_ps.tile([P, 512], f32, tag="vp")
                    for kk in range(16):
                        nc.tensor.matmul(
                            vp[:], wvt[:, kk, :], xcs[m2][:, kk, :],
                            start=(kk == 0), stop=(kk == 15),
                        )
                    nc.vector.tensor_copy(vt_mine[:, bass.ts(m2, 512)], vp[:])
                vnat_mine = pa_keep.tile([P, NSLOT, DV], f16, tag="vnm")
                for t in range(NSLOT):
                    vtp = pa_ps.tile([P, P], f32, tag="vtp")
                    nc.tensor.transpose(vtp[:], vt_mine[:, bass.ts(t, P)], id32[:])
                    nc.vector.tensor_copy(vnat_mine[:, t, :], vtp[:])
                cc_v_r = cc_v_in.rearrange("(t p) d -> p t d", p=P)
                for t4 in range(4):
                    nc.sync.dma_start(
                        out=cc_v_r[:, 2 * t4 : 2 * t4 + 2, :],
                        in_=vnat_mine[:, 2 * t4 : 2 * t4 + 2, :],
                    )
                nc.gpsimd.collective_compute(
                    "AllGather", OP.bypass, replica_groups=rg,
                    ins=[cc_v_in[:]], outs=[cc_v_out[:]],
                )

                # Q last: purely local, overlaps both gathers.
                for m2 in range(2):
                    qp = pa_ps.tile([P, 512], f32, tag="qp")
                    for kk in range(16):
                        nc.tensor.matmul(
                            qp[:], wqt[:, kk, :], xcs[m2][:, kk, :],
                            start=(kk == 0), stop=(kk == 15),
                        )
                    nc.vector.tensor_copy(qt[:, bass.ts(m2, 512)], qp[:])

            # ---------- phase B: load gathered K^T and V ----------
            # each 512-col chunk n covers ranks r0..r0+3 of one slot,
            # ascending — one DMA per chunk. K first (needed sooner).
            cc_k_r = cc_k_out.rearrange("(r p) m -> p r m", p=P)
            cc_v_r = cc_v_out.rearrange("(r s p) d -> p r s d", s=NSLOT, p=P)
            chunk_src = []
            for n in range(16):
                r0, s0 = _g_to_rank_slot(4 * n)
                for i in range(1, 4):
                    ri, si = _g_to_rank_slot(4 * n + i)
                    assert ri == r0 + i and si == s0
                chunk_src.append((r0, s0))
            for n, (r0, s0) in enumerate(chunk_src):
                nc.sync.dma_start(
                    out=kt_sb[:, bass.ts(n, 512)],
                    in_=cc_k_r[:, r0 : r0 + 4, s0 * P : (s0 + 1) * P],
                )
            for n, (r0, s0) in enumerate(chunk_src):
                nc.sync.dma_start(
                    out=vnat[:, 4 * n : 4 * n + 4, :],
                    in_=cc_v_r[:, r0 : r0 + 4, s0, :],
                )

            # ---------- phase C: attention ----------
            with (
                tc.tile_pool(name="stg", bufs=2) as stage_pool,
                tc.tile_pool(name="eb", bufs=3) as expb_pool,
                tc.tile_pool(name="stats", bufs=8) as stats,
                tc.tile_pool(name="atp", bufs=3) as atpool,
                tc.tile_pool(name="otp", bufs=2) as otpool,
                tc.tile_pool(name="osb", bufs=2) as osb_pool,
                tc.tile_pool(name="sps", bufs=2, space="PSUM") as spsum,
                tc.tile_pool(name="tps", bufs=2, space="PSUM") as tpsum,
                tc.tile_pool(name="ops", bufs=2, space="PSUM") as opsum,
            ):
                for sA, sB, E in GROUPS:
                    expb = {}
                    for slot in (sA, sB):
                        # score chunks in pairs: one 2-bank PSUM tile, one
                        # wide ACT copy (or DVE masked add), one dual-max.
                        Cs = C_SLOT[slot]
                        stg = stage_pool.tile([P, 16, 512], f32, tag="stg")
                        eb = expb_pool.tile([P, 16, 512], f16, tag="eb")
                        expb[slot] = eb
                        cmax = stats.tile([P, 16], f32, tag="cmax")
                        for h in range(Cs // 2):
                            n = 2 * h
                            sps = spsum.tile([P, 2, 512], f32, tag="sps")
                            for dn in range(2):
                                nc.tensor.matmul(
                                    sps[:, dn, :],
                                    qt[:, bass.ts(slot, P)],
                                    kt_sb[:, bass.ts(n + dn, 512)],
                                    start=True,
                                    stop=True,
                                )
                            if n == Cs - 2:
                                nc.vector.tensor_add(
                                    stg[:, n : n + 2, :], sps[:], mask_sb[:]
                                )
                                nc.vector.tensor_reduce(
                                    cmax[:, n : n + 2],
                                    stg[:, n : n + 2, :],
                                    axis=AX.X,
                                    op=OP.max,
                                )
                            else:
                                nc.scalar.copy(stg[:, n : n + 2, :], sps[:])
                                nc.vector.tensor_reduce(
                                    cmax[:, n : n + 2],
                                    sps[:],
                                    axis=AX.X,
                                    op=OP.max,
                                )
                        rmax = stats.tile([P, 1], f32, tag="rmax")
                        nc.vector.tensor_reduce(
                            rmax[:], cmax[:, :Cs], axis=AX.X, op=OP.max
                        )
                        negb = stats.tile([P, 1], f32, tag="negb")
                        nc.vector.tensor_scalar_mul(negb[:], rmax[:], -SCALE)
                        lsum = stats.tile([P, 1], f32, tag="lsum")
                        nc.scalar.activation(
                            out=eb[:, :Cs, :],
                            in_=stg[:, :Cs, :],
                            func=ACT.Exp,
                            bias=negb[:],
                            scale=SCALE,
                            accum_out=lsum[:],
                        )
                        nc.vector.reciprocal(rinv[:, slot : slot + 1], lsum[:])
                        if Cs < E:
                            nc.gpsimd.memset(eb[:, Cs:E, :], 0.0)

                    ebA, ebB = expb[sA], expb[sB]

                    def ebt(eb, j):
                        return eb[:, j // 4, (j % 4) * P : (j % 4 + 1) * P]

                    ops = opsum.tile([P, 256], f32, tag="ops")
                    for j2 in range(2 * E):
                        j = 2 * j2
                        tps = tpsum.tile([P, 4, P], f16, tag="tps")
                        nc.tensor.transpose(tps[:, 0, :], ebt(ebA, j), id16[:])
                        nc.tensor.transpose(tps[:, 1, :], ebt(ebB, j), id16[:])
                        nc.tensor.transpose(
                            tps[:, 2, :], ebt(ebA, j + 1), id16[:]
                        )
                        nc.tensor.transpose(
                            tps[:, 3, :], ebt(ebB, j + 1), id16[:]
                        )
                        at = atpool.tile([P, 4, P], f16, tag="at")
                        if j2 % 2 == 0:
                            nc.vector.tensor_copy(at[:], tps[:])
                        else:
                            nc.scalar.copy(at[:], tps[:])
                        nc.tensor.matmul(
                            ops[:],
                            vnat[:, j, :],
                            at[:, 0:2, :],
                            start=(j == 0),
                            stop=False,
                        )
                        nc.tensor.matmul(
                            ops[:],
                            vnat[:, j + 1, :],
                            at[:, 2:4, :],
                            start=False,
                            stop=(j + 2 == 4 * E),
                        )

                    ot = otpool.tile([P, 256], f32, tag="ot")
                    nc.vector.tensor_copy(ot[:], ops[:])
                    # ep tiles share the ops tag: the accumulator has been
                    # drained into ot by now, so the 2-buf rotation is safe.
                    for qq, slot in enumerate((sA, sB)):
                        ep = opsum.tile([P, 256], f32, tag="ops")
                        nc.tensor.transpose(
                            ep[:, :P], ot[:, bass.ts(qq, P)], id32[:]
                        )
                        o_sb = osb_pool.tile([P, DV], f32, tag="osb")
                        nc.vector.tensor_scalar_mul(
                            o_sb[:], ep[:, :P], rinv[:, slot : slot + 1]
                        )
                        nc.sync.dma_start(
                            out=out[bass.ts(slot, P), :], in_=o_sb[:]
                        )

    nc.compile()
    return nc


_NC_CACHE = None


def _get_nc():
    global _NC_CACHE
    if _NC_CACHE is None:
        _NC_CACHE = _build_nc()
    return _NC_CACHE


def _make_masks(c):
    # one mask per core, shared by all slots: with the g map above, the last
    # two chunks of every slot satisfy "keep iff flat_col <= p + 128*c".
    flat = np.arange(1024)[None, :]
    rows = np.arange(P)[:, None]
    m = np.where(flat <= rows + 128 * c, 0.0, NEG).astype(np.float32)
    return m.reshape(P, 2, 512)


def _prep_in_maps(x, w_q, w_k, w_v):
    x = np.ascontiguousarray(x, dtype=np.float32)
    w_q = np.ascontiguousarray(w_q, dtype=np.float32)
    w_k = np.ascontiguousarray(w_k, dtype=np.float32)
    w_v = np.ascontiguousarray(w_v, dtype=np.float32)
    in_maps = []
    for c in range(NCORES):
        rows = np.concatenate(
            [
                np.arange(_slot_to_g(c, s) * P, (_slot_to_g(c, s) + 1) * P)
                for s in range(NSLOT)
            ]
        )
        in_maps.append(
            {
                "xt": np.ascontiguousarray(x[rows].T),
                "mask": _make_masks(c),
                "wq": w_q,
                "wk": w_k,
                "wv": w_v,
            }
        )
    return in_maps


def _run(x, w_q, w_k, w_v, trace=False, trace_cores=None):
    nc = _get_nc()
    in_maps = _prep_in_maps(x, w_q, w_k, w_v)
    res = run_bass_kernel_spmd(
        nc,
        in_maps,
        list(range(NCORES)),
        trace=trace,
        trace_cores=trace_cores,
    )
    out = np.zeros((S, DV), dtype=np.float32)
    for c in range(NCORES):
        oc = res.results[c]["out"]
        for s in range(NSLOT):
            g = _slot_to_g(c, s)
            out[g * P : (g + 1) * P] = oc[s * P : (s + 1) * P]
    return out, res


def kernel(**inputs):
    out, _ = _run(inputs["x"], inputs["w_q"], inputs["w_k"], inputs["w_v"])
    return out
